# revision 9
# baseline (speedup 1.0000x reference)
"""Deformable Conv2d (K=3, stride 1, pad 1, dil 1) on 8 TRN2 NeuronCores.

Sharding: data-parallel over (batch=4) x (H halves=2) -> 8 cores.
Each core computes out[b, :, h0:h0+64, :] for its (b, h0).

v3 pipeline (dma_gather + bf16):
  1. offset conv (18ch) via PE f32 matmuls over a 1px-zero-padded image.
  2. PE-transpose offsets to point-major [128pts, 18].
  3. DVE coord math (f32): ys/xs, magic-number floor, frac (stored bf16),
     bilinear corner weights w00..w11 (bf16), int16 gather row indices
     into a 2px-zero-padded channels-last 4C-packed bf16 image in DRAM.
  4. idx shuffle to the SWDGE dma_gather wrapped layout (idx j read at
     partition 16 + j%16, halfword j//16 -- HW-probed): hop1 = 8 small
     SBUF->SBUF DMAs moving partition blocks, hop2 = 3 DVE int16 copies
     permuting free dims.
  5. Gather: 3x dma_gather (InstDMAGatherAnt, mlp gpsimd library) per
     2-group tile, NI=768 rows x 512B (2x2 px x 64ch bf16, 4C-packed).
     One Pool instruction generates 768 descriptors (vs 6 instructions
     of 128 each for indirect_dma_start); ucode ring caps NI at 1024.
  6. DVE bilinear lerp in bf16: s = w00*v00+w01*v01+w10*v10+w11*v11
     (7 wide ops per group).
  7. PE transpose S (bf16) to channel-major, main conv matmuls
     (K=576 as 4x128+64 accumulation, bf16), ACT bias add (f32), DMA out.
"""

import sys
for p in ("/opt/trn_rl_repo",):
    if p not in sys.path:
        sys.path.insert(0, p)

import numpy as np
import ml_dtypes

_BF = ml_dtypes.bfloat16

import concourse.bacc as bacc
import concourse.mybir as mybir
import concourse.tile as tile
import concourse.bass as bass
from concourse.bass_utils import run_bass_kernel_spmd
from concourse.library_config import mlp as _mlp_lib

F32 = mybir.dt.float32
BF16 = mybir.dt.bfloat16
I16 = mybir.dt.int16
AL = mybir.AluOpType
AF = mybir.ActivationFunctionType

B, C, H, W = 4, 64, 128, 128
K, KK = 3, 9
O = 64                      # output channels
OC = 2 * KK                 # offset channels (18)
HL = H // 2                 # local rows per core (64)
NPT = HL * W                # local points per core (8192)
NG = NPT // 128             # point groups of 128 (=64); group g == local row g
W2 = W + 2                  # 1px-padded width for offset conv (130)
H2 = HL + 2                 # 1px-padded local rows (66)
W4 = W + 4                  # 2px-padded width for gather image (132)
H4 = H + 4                  # 2px-padded height (full image!) (132)
MAGIC = float(3 * 2 ** 22)   # 1.5*2^23: ulp stays 1.0 for f32 in [-2^22, 2^22]
GCH = 2                     # point-groups per gather tile
E = 256                     # gathered elems per (point, tap): 2x2 px x 64ch
GBLK = KK * E               # gathered elems per point per group (2304)
NB = GCH * KK               # blocks per gather tile (18)
NCALL = 3                   # dma_gather calls per tile (6 blocks each)
NI = (NB // NCALL) * 128    # idxs per dma_gather call (768; ucode cap 1024)
CHUNKS = 4                  # preamble chunks (groups per chunk = NG/CHUNKS)
CG = NG // CHUNKS           # groups per chunk (16)
NT = CG // GCH              # gather tiles per chunk (8)
WCOL = CG * KK * 8          # wrapped idx cols per chunk (1152)


def build_program(dbg=False, skip_gather=False, skip_lerp=False,
                  skip_mm=False, skip_off=False, reps=1):
    nc = bacc.Bacc("TRN2", target_bir_lowering=False, debug=False)

    xp = nc.dram_tensor("xp", [C, H2 * W2], F32, kind="ExternalInput")
    xcl = nc.dram_tensor("xcl", [H4 * W4, 4 * C], BF16, kind="ExternalInput")
    wofft = nc.dram_tensor("wofft", [C, KK * OC], F32, kind="ExternalInput")
    woffb = nc.dram_tensor("woffb", [OC, 1], F32, kind="ExternalInput")
    wmain = nc.dram_tensor("wmain", [128, 5 * O], BF16, kind="ExternalInput")
    wb = nc.dram_tensor("wb", [O, 1], F32, kind="ExternalInput")
    basey = nc.dram_tensor("basey", [128, NG * KK], F32, kind="ExternalInput")
    basex = nc.dram_tensor("basex", [128, NG * KK], F32, kind="ExternalInput")
    ident = nc.dram_tensor("ident", [128, 128], F32, kind="ExternalInput")
    identb = nc.dram_tensor("identb", [128, 128], BF16, kind="ExternalInput")
    out = nc.dram_tensor("out", [O, NPT], F32, kind="ExternalOutput")

    with tile.TileContext(nc) as tc:
        with (
            tc.tile_pool(name="cst", bufs=1) as cst,
            tc.tile_pool(name="psA", bufs=2, space="PSUM") as psA,
            tc.tile_pool(name="psT", bufs=2, space="PSUM") as psT,
            tc.tile_pool(name="psO", bufs=2, space="PSUM") as psO,
        ):
            nc.gpsimd.load_library(_mlp_lib)
            # ---- load constants / weights (once) ----
            ident_t = cst.tile([128, 128], F32, tag="ident")
            nc.sync.dma_start(out=ident_t[:], in_=ident[:])
            identb_t = cst.tile([128, 128], BF16, tag="identb")
            nc.sync.dma_start(out=identb_t[:], in_=identb[:])
            wofft_t = cst.tile([C, KK * OC], F32, tag="wofft")
            nc.sync.dma_start(out=wofft_t[:], in_=wofft[:])
            woffb_t = cst.tile([OC, 1], F32, tag="woffb")
            nc.sync.dma_start(out=woffb_t[:], in_=woffb[:])
            wmain_t = cst.tile([128, 5 * O], BF16, tag="wmain")
            nc.sync.dma_start(out=wmain_t[:], in_=wmain[:])
            wb_t = cst.tile([O, 1], F32, tag="wb")
            nc.sync.dma_start(out=wb_t[:], in_=wb[:])
            basey_t = cst.tile([128, NG * KK], F32, tag="basey")
            nc.sync.dma_start(out=basey_t[:], in_=basey[:])
            basex_t = cst.tile([128, NG * KK], F32, tag="basex")
            nc.sync.dma_start(out=basex_t[:], in_=basex[:])

            for rep in range(reps):
                build_body(nc, tc, psA, psT, psO, rep,
                           ident_t, identb_t, wofft_t, woffb_t, wmain_t,
                           wb_t, basey_t, basex_t, xp, xcl, out,
                           skip_gather, skip_lerp, skip_mm, skip_off)

    nc.compile()
    return nc


def build_body(nc, tc, psA, psT, psO, rep,
               ident_t, identb_t, wofft_t, woffb_t, wmain_t, wb_t,
               basey_t, basex_t, xp, xcl, out,
               skip_gather, skip_lerp, skip_mm, skip_off):
    with (
        tc.tile_pool(name=f"early{rep}", bufs=1) as early,
        tc.tile_pool(name=f"coord{rep}", bufs=1) as coord,
        tc.tile_pool(name=f"ctmp{rep}", bufs=2) as ctmp,
        tc.tile_pool(name=f"gat{rep}", bufs=3) as gat,
        tc.tile_pool(name=f"lrp{rep}", bufs=2) as lrp,
        tc.tile_pool(name=f"outp{rep}", bufs=1) as outp,
    ):
        xp_t = early.tile([C, H2 * W2], F32, tag="xp")
        nc.sync.dma_start(out=xp_t[:], in_=xp[:])
        xp3 = xp_t[:].rearrange("c (h w) -> c h w", h=H2)

        # per-chunk coord tiles so chunk-0 gathers don't wait on chunk-3
        w_c = []      # per chunk: [w00, w01, w10, w11] bf16 [128, CG*KK]
        wrap_c = []   # per chunk: wrapped int16 idx [128, WCOL]
        for ch in range(CHUNKS):
            w_c.append([coord.tile([128, CG * KK], BF16, tag=f"w{q}{ch}",
                                   name=f"w{q}{ch}_{rep}") for q in range(4)])
            wrap_c.append(coord.tile([128, WCOL], I16, tag=f"wr{ch}",
                                     name=f"wr{ch}_{rep}"))
        off_c = [coord.tile([OC, CG * W], F32, tag=f"off{ch}",
                            name=f"off{ch}_{rep}")
                 for ch in range(CHUNKS)]
        out_sb = outp.tile([O, NPT], F32, tag="osb")
        if skip_mm:
            nc.vector.memset(out_sb[:], 0.0)

        def preamble_chunk(ch):
            g0 = ch * CG          # first group (= local row) of chunk
            off_t = off_c[ch]
            if skip_off:
                nc.vector.memset(off_t[:], 0.0)
            # ---- offset conv rows [g0, g0+CG) ----
            RPC = 4               # rows per psum chunk (N=512)
            for r0 in ([] if skip_off else range(g0, g0 + CG, RPC)):
                ps = psA.tile([OC, RPC * W], F32, tag="psA")
                for kk in range(KK):
                    ki, kj = kk // K, kk % K
                    rhs = xp3[:, r0 + ki:r0 + ki + RPC, kj:kj + W]
                    nc.tensor.matmul(
                        out=ps[:], lhsT=wofft_t[:, kk * OC:(kk + 1) * OC],
                        rhs=rhs, start=(kk == 0), stop=(kk == KK - 1))
                nc.scalar.activation(
                    out=off_t[:, (r0 - g0) * W:(r0 - g0 + RPC) * W], in_=ps[:],
                    func=AF.Identity, bias=woffb_t[:, 0:1], scale=1.0)

            # ---- transpose offsets to point-major [128, CG*18] ----
            offT = ctmp.tile([128, CG * OC], F32, tag="offT")
            for gl in range(CG):
                ps = psT.tile([128, OC], F32, tag="psT")
                nc.tensor.transpose(
                    out=ps[:], in_=off_t[:, gl * 128:(gl + 1) * 128],
                    identity=ident_t[:OC, :OC])
                nc.scalar.copy(out=offT[:, gl * OC:(gl + 1) * OC], in_=ps[:])

            # ---- coordinate math (wide [128, CG*KK] f32 ops) ----
            NW = CG * KK
            o4 = offT[:].rearrange("p (g k t) -> p g k t", g=CG, k=KK)
            dy = o4[:, :, :, 0]
            dx = o4[:, :, :, 1]

            ys = ctmp.tile([128, NW], F32, tag="ys")
            xs = ctmp.tile([128, NW], F32, tag="xs")
            rr = ctmp.tile([128, NW], F32, tag="rr")
            mm_ = ctmp.tile([128, NW], F32, tag="mm")
            y0 = ctmp.tile([128, NW], F32, tag="y0")
            x0 = ctmp.tile([128, NW], F32, tag="x0")
            ti = ctmp.tile([128, NW], F32, tag="ti")
            fy = ctmp.tile([128, NW], BF16, tag="fy")
            fx = ctmp.tile([128, NW], BF16, tag="fx")
            idx16 = ctmp.tile([128, NW], I16, tag="idx16")
            wrapv1 = ctmp.tile([128, WCOL], I16, tag="wrapv1")

            ys3 = ys[:].rearrange("p (g k) -> p g k", g=CG)
            xs3 = xs[:].rearrange("p (g k) -> p g k", g=CG)
            by3 = basey_t[:, g0 * KK:(g0 + CG) * KK].rearrange(
                "p (g k) -> p g k", g=CG)
            bx3 = basex_t[:, g0 * KK:(g0 + CG) * KK].rearrange(
                "p (g k) -> p g k", g=CG)
            nc.vector.tensor_tensor(out=ys3, in0=dy, in1=by3, op=AL.add)
            nc.vector.tensor_tensor(out=xs3, in0=dx, in1=bx3, op=AL.add)

            def floorv(src, dst, frac):
                # magic-number round-to-nearest, then fix round-ups
                nc.vector.tensor_scalar(
                    out=rr[:], in0=src[:], scalar1=MAGIC, scalar2=MAGIC,
                    op0=AL.add, op1=AL.subtract)
                nc.vector.tensor_tensor(out=mm_[:], in0=rr[:], in1=src[:],
                                        op=AL.is_gt)
                nc.vector.tensor_tensor(out=dst[:], in0=rr[:], in1=mm_[:],
                                        op=AL.subtract)
                nc.vector.tensor_tensor(out=frac[:], in0=src[:], in1=dst[:],
                                        op=AL.subtract)

            floorv(ys, y0, fy)
            floorv(xs, x0, fx)
            # corner weights in bf16: w00=(1-fy)(1-fx), w01=(1-fy)fx,
            # w10=fy(1-fx), w11=fy*fx
            w00, w01, w10, w11 = w_c[ch]
            gy = ctmp.tile([128, NW], BF16, tag="gy")   # 1-fy
            gx = ctmp.tile([128, NW], BF16, tag="gx")   # 1-fx
            # gy = (fy - 1) * -1 = 1 - fy; same for gx
            nc.vector.tensor_scalar(out=gy[:], in0=fy[:], scalar1=1.0,
                                    scalar2=-1.0, op0=AL.subtract,
                                    op1=AL.mult)
            nc.vector.tensor_scalar(out=gx[:], in0=fx[:], scalar1=1.0,
                                    scalar2=-1.0, op0=AL.subtract,
                                    op1=AL.mult)
            nc.vector.tensor_tensor(out=w00[:], in0=gy[:], in1=gx[:],
                                    op=AL.mult)
            nc.vector.tensor_tensor(out=w01[:], in0=gy[:], in1=fx[:],
                                    op=AL.mult)
            nc.vector.tensor_tensor(out=w10[:], in0=fy[:], in1=gx[:],
                                    op=AL.mult)
            nc.vector.tensor_tensor(out=w11[:], in0=fy[:], in1=fx[:],
                                    op=AL.mult)
            # clamp (reuse rr/mm as clamped outputs)
            nc.vector.tensor_scalar(out=rr[:], in0=y0[:], scalar1=-2.0,
                                    scalar2=float(H), op0=AL.max, op1=AL.min)
            nc.vector.tensor_scalar(out=mm_[:], in0=x0[:], scalar1=-2.0,
                                    scalar2=float(W), op0=AL.max, op1=AL.min)
            # idx = (y0c*W4 + x0c) + (2*W4+2), int16
            nc.vector.scalar_tensor_tensor(
                out=ti[:], in0=rr[:], scalar=float(W4), in1=mm_[:],
                op0=AL.mult, op1=AL.add)
            nc.vector.tensor_scalar(
                out=idx16[:], in0=ti[:], scalar1=float(2 * W4 + 2),
                scalar2=None, op0=AL.add)

            # ---- shuffle idx16 [128 pts, NW] -> SWDGE wrapped layout ----
            # target cell for gather-call-local idx j (tile t, call c,
            # block b, point p=16r+q): (16+q, t*144 + c*48 + b*8 + r)
            # hop1: partition fold p=16r+q -> partition q, col r*NW+col
            for r in range(8):
                nc.sync.dma_start(
                    out=wrapv1[0:16, r * NW:(r + 1) * NW],
                    in_=idx16[16 * r:16 * r + 16, 0:NW])
            # hop2 (DVE, partitions 0..15): free-dim permutation
            # (r, t, c, b) -> (t, c, b, r)
            wrapv2 = ctmp.tile([128, WCOL], I16, tag="wrapv2")
            iv = wrapv1[0:16, :].rearrange(
                "p (r t c b) -> p t c b r", r=8, t=NT, c=NCALL)
            ov = wrapv2[0:16, :].rearrange(
                "p (t c b r) -> p t c b r", t=NT, c=NCALL, b=NB // NCALL)
            for c in range(NCALL):
                nc.vector.tensor_scalar(
                    out=ov[:, :, c, :, :], in0=iv[:, :, c, :, :],
                    scalar1=0, scalar2=None, op0=AL.add)
            # hop3: shift to partitions 16..31 where the gather ucode
            # reads (idx j at partition 16 + j%16, halfword j//16)
            nc.sync.dma_start(
                out=wrap_c[ch][16:32, :], in_=wrapv2[0:16, :])

        def main_groups(ch):
            g0 = ch * CG
            for c0 in range(g0, g0 + CG, GCH):
                t = (c0 - g0) // GCH      # tile index within chunk
                gt = gat.tile([128, GCH * GBLK], BF16, tag="G")
                if skip_gather and not skip_lerp:
                    nc.vector.memset(gt[:], 0.0)
                # 3 dma_gather calls of 6 blocks (768 idxs) each: one Pool
                # instruction generates 768 descriptors of 512B.
                for c in ([] if skip_gather else range(NCALL)):
                    nc.gpsimd.dma_gather(
                        out_ap=gt[:, c * 6 * E:(c + 1) * 6 * E].rearrange(
                            "p (g e) -> p g e", e=E),
                        in_ap=xcl[:],
                        idxs_ap=wrap_c[ch][:, t * 144 + c * 48:
                                           t * 144 + (c + 1) * 48],
                        num_idxs=NI, num_idxs_reg=NI, elem_size=E,
                        queue_num=0)
                for gs in range(GCH):
                    g = c0 + gs
                    gl = g - g0
                    g5 = gt[:, gs * GBLK:(gs + 1) * GBLK].rearrange(
                        "p (k r q c) -> p k r q c", k=KK, r=2, q=2)
                    v00 = g5[:, :, 0, 0, :]
                    v01 = g5[:, :, 0, 1, :]
                    v10 = g5[:, :, 1, 0, :]
                    v11 = g5[:, :, 1, 1, :]

                    def wb_(wt):
                        return wt[:, gl * KK:(gl + 1) * KK].unsqueeze(2) \
                            .to_broadcast([128, KK, C])

                    d_ = lrp.tile([128, KK * C], BF16, tag="d")
                    m_ = lrp.tile([128, KK * C], BF16, tag="m")
                    l0 = lrp.tile([128, KK * C], BF16, tag="l0")
                    l1 = lrp.tile([128, KK * C], BF16, tag="l1")
                    s_ = lrp.tile([128, KK * C], BF16, tag="s")
                    if skip_lerp and not skip_mm:
                        nc.vector.memset(s_[:], 0.0)
                    d3 = d_[:].rearrange("p (k c) -> p k c", k=KK)
                    m3 = m_[:].rearrange("p (k c) -> p k c", k=KK)
                    l03 = l0[:].rearrange("p (k c) -> p k c", k=KK)
                    l13 = l1[:].rearrange("p (k c) -> p k c", k=KK)
                    s3 = s_[:].rearrange("p (k c) -> p k c", k=KK)

                    if not skip_lerp:
                        w00, w01, w10, w11 = w_c[ch]
                        nc.vector.tensor_tensor(out=d3, in0=v00,
                                                in1=wb_(w00), op=AL.mult)
                        nc.vector.tensor_tensor(out=m3, in0=v01,
                                                in1=wb_(w01), op=AL.mult)
                        nc.vector.tensor_tensor(out=l03, in0=d3, in1=m3,
                                                op=AL.add)
                        nc.vector.tensor_tensor(out=d3, in0=v10,
                                                in1=wb_(w10), op=AL.mult)
                        nc.vector.tensor_tensor(out=m3, in0=v11,
                                                in1=wb_(w11), op=AL.mult)
                        nc.vector.tensor_tensor(out=l13, in0=d3, in1=m3,
                                                op=AL.add)
                        nc.vector.tensor_tensor(out=s3, in0=l03, in1=l13,
                                                op=AL.add)

                    if skip_mm:
                        continue
                    st = lrp.tile([128, 640], BF16, tag="st")
                    for j in range(4):
                        ps = psT.tile([128, 128], BF16, tag="psTb")
                        nc.tensor.transpose(
                            out=ps[:], in_=s_[:, j * 128:(j + 1) * 128],
                            identity=identb_t[:])
                        nc.scalar.copy(out=st[:, j * 128:(j + 1) * 128],
                                       in_=ps[:])
                    ps = psT.tile([64, 128], BF16, tag="psTb")
                    nc.tensor.transpose(
                        out=ps[:], in_=s_[:, 512:576], identity=identb_t[:])
                    nc.scalar.copy(out=st[:64, 512:640], in_=ps[:])

                    po = psO.tile([O, 128], F32, tag="psO")
                    for j in range(4):
                        nc.tensor.matmul(
                            out=po[:], lhsT=wmain_t[:, j * O:(j + 1) * O],
                            rhs=st[:, j * 128:(j + 1) * 128],
                            start=(j == 0), stop=False)
                    nc.tensor.matmul(
                        out=po[:], lhsT=wmain_t[:64, 4 * O:5 * O],
                        rhs=st[:64, 512:640], start=False, stop=True)
                    nc.scalar.activation(
                        out=out_sb[:, g * 128:(g + 1) * 128], in_=po[:],
                        func=AF.Identity, bias=wb_t[:, 0:1], scale=1.0)
            # out DMA per chunk (overlaps with next chunk's work)
            nc.sync.dma_start(
                out=out[:, g0 * 128:(g0 + CG) * 128],
                in_=out_sb[:, g0 * 128:(g0 + CG) * 128])

        # chunk 0 preamble, then stream: gathers(ch) while preamble(ch+1)
        preamble_chunk(0)
        for ch in range(CHUNKS):
            if ch + 1 < CHUNKS:
                preamble_chunk(ch + 1)
            main_groups(ch)


_NC_CACHE = None


def _get_nc():
    global _NC_CACHE
    if _NC_CACHE is None:
        _NC_CACHE = build_program()
    return _NC_CACHE


def make_core_inputs(x, weight, bias, offset_w, offset_b):
    """Host-side prep: returns list of 8 in_maps (core i = batch i//2, half i%2)."""
    x = np.asarray(x, np.float32)
    weight = np.asarray(weight, np.float32)
    bias = np.asarray(bias, np.float32)
    offset_w = np.asarray(offset_w, np.float32)
    offset_b = np.asarray(offset_b, np.float32)

    xp_full = np.pad(x, ((0, 0), (0, 0), (1, 1), (1, 1)))
    xpad = np.pad(x, ((0, 0), (0, 0), (2, 2), (2, 3)))  # extra right col for x+1
    xpad = np.pad(xpad, ((0, 0), (0, 0), (0, 1), (0, 0)))  # extra bottom row
    xcl0 = xpad.transpose(0, 2, 3, 1)           # [B, 133, 133, C]
    zz = np.empty((B, H4, W4, 4 * C), np.float32)
    zz[..., 0 * C:1 * C] = xcl0[:, :H4, :W4, :]
    zz[..., 1 * C:2 * C] = xcl0[:, :H4, 1:W4 + 1, :]
    zz[..., 2 * C:3 * C] = xcl0[:, 1:H4 + 1, :W4, :]
    zz[..., 3 * C:4 * C] = xcl0[:, 1:H4 + 1, 1:W4 + 1, :]
    xcl_full = zz.astype(_BF)

    # offset conv weights: [c, kk*18], lhsT per tap
    wofft = np.ascontiguousarray(
        offset_w.reshape(OC, C, KK).transpose(1, 2, 0)).reshape(C, KK * OC)
    woffb = offset_b.reshape(OC, 1)
    # main conv weights: [128, 5*64]; block j rows (t2*64+c), cols o
    wr = weight.reshape(O, C, KK)
    wmain = np.zeros((128, 5 * O), np.float32)
    for j in range(5):
        for t2 in range(2):
            kk = 2 * j + t2
            if kk >= KK:
                break
            wmain[t2 * C:(t2 + 1) * C, j * O:(j + 1) * O] = wr[:, :, kk].T
    wmain = wmain.astype(_BF)
    wb = bias.reshape(O, 1)
    identm = np.eye(128, dtype=np.float32)
    identb = np.eye(128, dtype=np.float32).astype(_BF)

    p = np.arange(128, dtype=np.float32)
    g = np.arange(NG, dtype=np.float32)
    kki = (np.arange(KK) // K).astype(np.float32)
    kkj = (np.arange(KK) % K).astype(np.float32)
    # basex[p, g, kk] = p - 1 + kj
    basex = (p[:, None, None] - 1.0 + kkj[None, None, :]) \
        + 0.0 * g[None, :, None]
    basex = np.ascontiguousarray(
        np.broadcast_to(basex, (128, NG, KK)), np.float32).reshape(128, NG * KK)

    in_maps = []
    for core in range(8):
        b, h0 = core // 2, (core % 2) * HL
        by = np.broadcast_to(
            (h0 + g)[None, :, None] - 1.0 + kki[None, None, :],
            (128, NG, KK))
        in_maps.append({
            "xp": np.ascontiguousarray(
                xp_full[b, :, h0:h0 + H2, :]).reshape(
                    C, H2 * W2),
            "xcl": np.ascontiguousarray(xcl_full[b]).reshape(H4 * W4, 4 * C),
            "wofft": wofft, "woffb": woffb,
            "wmain": wmain, "wb": wb,
            "basey": np.ascontiguousarray(by, np.float32).reshape(128, NG * KK),
            "basex": basex,
            "ident": identm, "identb": identb,
        })
    return in_maps


def kernel(x, weight, bias, offset_w, offset_b):
    nc = _get_nc()
    in_maps = make_core_inputs(x, weight, bias, offset_w, offset_b)
    res = run_bass_kernel_spmd(nc, in_maps, list(range(8)))
    out_full = np.empty((B, O, H, W), np.float32)
    for core in range(8):
        b, h0 = core // 2, (core % 2) * HL
        out_full[b, :, h0:h0 + HL, :] = res.results[core]["out"].reshape(O, HL, W)
    return out_full
